# revision 5
# baseline (speedup 1.0000x reference)
"""Contrastive loss kernel for Trainium2 (8 NeuronCores, batch-parallel).

Problem (hardcoded):
  X: (32, 16384, 256) f32   pair embeddings, e_a = X[..., :128], e_b = X[..., 128:]
  y: (32, 128, 128)  i32    adjacency in {0, 1}
  out: (32, 16384)   f32    where(y==1, dist2, relu(1 - dist2))

Sharding: data-parallel over batch, 4 batches per core, no communication.
"""

from contextlib import ExitStack

import numpy as np

import concourse.bass as bass
import concourse.tile as tile
from concourse import bacc, masks, mybir
from concourse.bass_utils import run_bass_kernel_spmd

F32 = mybir.dt.float32
I32 = mybir.dt.int32

B, P, D = 32, 16384, 256
H = D // 2  # 128
ALPHA_MARGIN = 1.0
N_CORES = 8
BPC = B // N_CORES  # batches per core

PART = 128           # SBUF partitions; also pairs per result column
SLOTS = 8            # pair-columns per big tile
TILES = P // (PART * SLOTS)  # big tiles per batch (16)


def build_program(bpc=BPC, slots=SLOTS, tiles=None, pairs=P):
    """Build the per-core Bass program. Shapes are per-core (full batch dim / 8)."""
    if tiles is None:
        tiles = pairs // (PART * slots)
    assert tiles * slots * PART == pairs
    ncols = tiles * slots  # result columns per batch (pairs // 128)

    nc = bacc.Bacc("TRN2", target_bir_lowering=False, debug=False,
                   num_devices=N_CORES)
    X = nc.dram_tensor("X", [bpc, pairs, D], F32, kind="ExternalInput").ap()
    Y = nc.dram_tensor("y", [bpc, pairs], I32, kind="ExternalInput").ap()
    O = nc.dram_tensor("out", [bpc, pairs], F32, kind="ExternalOutput").ap()

    with tile.TileContext(nc) as tc, ExitStack() as ctx:
        xpool = ctx.enter_context(tc.tile_pool(name="x", bufs=3))
        dpool = ctx.enter_context(tc.tile_pool(name="diff", bufs=3))
        rpool = ctx.enter_context(tc.tile_pool(name="res", bufs=2))
        ppool = ctx.enter_context(tc.tile_pool(name="psum", bufs=2, space="PSUM"))
        spool = ctx.enter_context(tc.tile_pool(name="small", bufs=2))
        cpool = ctx.enter_context(tc.tile_pool(name="const", bufs=1))

        ident = cpool.tile([PART, PART], F32)
        masks.make_identity(nc, ident[:])
        ones = cpool.tile([PART, 1], F32)
        nc.gpsimd.memset(ones[:], 1.0)

        for b in range(bpc):
            # pair index = t*128 + p  ->  [p, t, f] view of X[b]
            Xb = X[b].rearrange("(t p) f -> p t f", p=PART)
            res = rpool.tile([PART, ncols], F32)
            for g in range(tiles):
                xt = xpool.tile([PART, slots, D], F32)
                nc.sync.dma_start(xt[:], Xb[:, g * slots:(g + 1) * slots, :])
                dft = dpool.tile([PART, slots, H], F32)
                nc.vector.tensor_sub(dft[:], xt[:, :, 0:H], xt[:, :, H:D])
                for j in range(slots):
                    c = g * slots + j
                    col = res[:, c:c + 1]
                    if j % 2 == 0:
                        # ACT: out = Square(diff), accum_out = sum -> dist2
                        nc.scalar.activation(
                            dft[:, j, :], dft[:, j, :],
                            mybir.ActivationFunctionType.Square,
                            accum_out=col,
                        )
                    else:
                        # DVE: out = diff * diff, accum_out = sum -> dist2
                        nc.vector.scalar_tensor_tensor(
                            out=dft[:, j, :], in0=dft[:, j, :], scalar=0.0,
                            in1=dft[:, j, :],
                            op0=mybir.AluOpType.bypass, op1=mybir.AluOpType.mult,
                            accum_out=col,
                        )

            # res[p, t] = dist2(pair t*128+p); transpose so partition = t
            pres = ppool.tile([ncols, PART], F32)
            nc.tensor.transpose(pres[:], res[:], ident[:])

            yt = spool.tile([ncols, PART], I32)
            nc.sync.dma_start(yt[:], Y[b].rearrange("(t p) -> t p", p=PART))

            # neg = relu(margin - dist2)
            neg = spool.tile([ncols, PART], F32)
            nc.scalar.activation(
                neg[:], pres[:], mybir.ActivationFunctionType.Relu,
                scale=-1.0, bias=ones[0:ncols, 0:1],
            )
            outt = spool.tile([ncols, PART], F32)
            nc.vector.tensor_copy(outt[:], neg[:])
            nc.vector.copy_predicated(outt[:], yt[:], pres[:])

            nc.sync.dma_start(O[b].rearrange("(t p) -> t p", p=PART), outt[:])

    nc.compile()
    return nc


_PROGRAM_CACHE = {}


def _get_program():
    if "nc" not in _PROGRAM_CACHE:
        _PROGRAM_CACHE["nc"] = build_program()
    return _PROGRAM_CACHE["nc"]


def kernel(X, y):
    X = np.asarray(X, dtype=np.float32)
    y = np.asarray(y, dtype=np.int32).reshape(B, P)
    assert X.shape == (B, P, D)

    nc = _get_program()
    in_maps = [
        {"X": np.ascontiguousarray(X[c * BPC:(c + 1) * BPC]),
         "y": np.ascontiguousarray(y[c * BPC:(c + 1) * BPC])}
        for c in range(N_CORES)
    ]
    res = run_bass_kernel_spmd(nc, in_maps, list(range(N_CORES)))
    out = np.concatenate([res.results[c]["out"] for c in range(N_CORES)], axis=0)
    return out.astype(np.float32)


# revision 7
# speedup vs baseline: 4.1604x; 4.1604x over previous
"""Contrastive loss kernel for Trainium2 (8 NeuronCores, batch-parallel).

Problem (hardcoded):
  X: (32, 16384, 256) f32   pair embeddings, e_a = X[..., :128], e_b = X[..., 128:]
  y: (32, 128, 128)  i32    adjacency in {0, 1}
  out: (32, 16384)   f32    where(y==1, dist2, relu(1 - dist2))

Sharding: data-parallel over batch, 4 batches per core, no communication.
"""

from contextlib import ExitStack

import numpy as np

import concourse.bass as bass
import concourse.tile as tile
from concourse import bacc, masks, mybir
from concourse.bass_utils import run_bass_kernel_spmd

F32 = mybir.dt.float32
I32 = mybir.dt.int32

B, P, D = 32, 16384, 256
H = D // 2  # 128
ALPHA_MARGIN = 1.0
N_CORES = 8
BPC = B // N_CORES  # batches per core

PART = 128           # SBUF partitions; also pairs per result column
SLOTS = 8            # pair-columns per big tile
TILES = P // (PART * SLOTS)  # big tiles per batch (16)


def build_program(bpc=BPC, slots=SLOTS, tiles=None, pairs=P, passes=1):
    """Build the per-core Bass program. Shapes are per-core (full batch dim / 8).

    passes>1 repeats the whole computation (idempotent) — used only for
    marginal-time benchmarking, never for the graded kernel."""
    if tiles is None:
        tiles = pairs // (PART * slots)
    assert tiles * slots * PART == pairs
    ncols = tiles * slots  # result columns per batch (pairs // 128)

    nc = bacc.Bacc("TRN2", target_bir_lowering=False, debug=False,
                   num_devices=N_CORES)
    X = nc.dram_tensor("X", [bpc, pairs, D], F32, kind="ExternalInput").ap()
    Y = nc.dram_tensor("y", [bpc, pairs], I32, kind="ExternalInput").ap()
    O = nc.dram_tensor("out", [bpc, pairs], F32, kind="ExternalOutput").ap()

    with tile.TileContext(nc) as tc, ExitStack() as ctx:
        xpool = ctx.enter_context(tc.tile_pool(name="x", bufs=3))
        dpool = ctx.enter_context(tc.tile_pool(name="diff", bufs=3))
        rpool = ctx.enter_context(tc.tile_pool(name="res", bufs=2))
        ppool = ctx.enter_context(tc.tile_pool(name="psum", bufs=2, space="PSUM"))
        spool = ctx.enter_context(tc.tile_pool(name="small", bufs=2))
        cpool = ctx.enter_context(tc.tile_pool(name="const", bufs=1))

        ident = cpool.tile([PART, PART], F32)
        masks.make_identity(nc, ident[:])
        ones = cpool.tile([PART, 1], F32)
        nc.gpsimd.memset(ones[:], 1.0)

        for b in [b for _ in range(passes) for b in range(bpc)]:
            # pair index = t*128 + p  ->  [p, t, f] view of X[b]
            Xb = X[b].rearrange("(t p) f -> p t f", p=PART)
            res = rpool.tile([PART, ncols], F32)
            for g in range(tiles):
                xt = xpool.tile([PART, slots, D], F32)
                nc.sync.dma_start(xt[:], Xb[:, g * slots:(g + 1) * slots, :])
                dft = dpool.tile([PART, slots, H], F32)
                nc.vector.tensor_sub(dft[:], xt[:, :, 0:H], xt[:, :, H:D])
                for j in range(slots):
                    c = g * slots + j
                    col = res[:, c:c + 1]
                    if j % 2 == 0:
                        # ACT: out = Square(diff), accum_out = sum -> dist2
                        nc.scalar.activation(
                            dft[:, j, :], dft[:, j, :],
                            mybir.ActivationFunctionType.Square,
                            accum_out=col,
                        )
                    else:
                        # DVE: out = diff * diff, accum_out = sum -> dist2
                        nc.vector.scalar_tensor_tensor(
                            out=dft[:, j, :], in0=dft[:, j, :], scalar=0.0,
                            in1=dft[:, j, :],
                            op0=mybir.AluOpType.bypass, op1=mybir.AluOpType.mult,
                            accum_out=col,
                        )

            # res[p, t] = dist2(pair t*128+p); transpose so partition = t
            pres = ppool.tile([ncols, PART], F32)
            nc.tensor.transpose(pres[:], res[:], ident[:])

            yt = spool.tile([ncols, PART], I32)
            nc.sync.dma_start(yt[:], Y[b].rearrange("(t p) -> t p", p=PART))

            # neg = relu(margin - dist2)
            neg = spool.tile([ncols, PART], F32)
            nc.scalar.activation(
                neg[:], pres[:], mybir.ActivationFunctionType.Relu,
                scale=-1.0, bias=ones[0:ncols, 0:1],
            )
            outt = spool.tile([ncols, PART], F32)
            nc.vector.tensor_copy(outt[:], neg[:])
            nc.vector.copy_predicated(outt[:], yt[:], pres[:])

            nc.sync.dma_start(O[b].rearrange("(t p) -> t p", p=PART), outt[:])

    nc.compile()
    return nc


_PROGRAM_CACHE = {}


def _get_program():
    if "nc" not in _PROGRAM_CACHE:
        _PROGRAM_CACHE["nc"] = build_program()
    return _PROGRAM_CACHE["nc"]


def kernel(X, y):
    X = np.asarray(X, dtype=np.float32)
    y = np.asarray(y, dtype=np.int32).reshape(B, P)
    assert X.shape == (B, P, D)

    nc = _get_program()
    in_maps = [
        {"X": np.ascontiguousarray(X[c * BPC:(c + 1) * BPC]),
         "y": np.ascontiguousarray(y[c * BPC:(c + 1) * BPC])}
        for c in range(N_CORES)
    ]
    res = run_bass_kernel_spmd(nc, in_maps, list(range(N_CORES)))
    out = np.concatenate([res.results[c]["out"] for c in range(N_CORES)], axis=0)
    return out.astype(np.float32)
